# revision 25
# baseline (speedup 1.0000x reference)
"""Distributed multi-head attention kernel for 8 Trainium2 NeuronCores.

Problem: x[4,2048,1024] -> qkv proj -> 16-head attention (add_zero_attn)
         -> out proj + bias -> [4,2048,1024]

Sharding: 8 cores = 4 batches x 2 query-halves. Each core computes the
full K/V for its batch (KV projection duplicated across the pair, ~4GFLOP,
far cheaper than any 2-rank collective on this fabric) and attention +
output projection for its own 1024 queries. Zero collectives; host
reassembles by concatenation only.

add_zero_attn appends a zero key & value token: the value row is zero so it
only adds +1 to each softmax denominator. We therefore never materialize it;
denominators come from a ones-column appended to V (column 64 of each head's
v tile) and get +1 before the reciprocal.

Math per core (all matmuls in bf16, accumulation f32):
  xT   = transpose(x)                      via XBAR DMA-transpose (bf16)
  qT   = W_q^T x_q^T   [1024, 1024]        (inner on partitions)
  kT   = W_k^T x^T     [1024, 2048]
  v    = x W_v         [2048, 1024(+ones)] (kpos on partitions)
  per head pair, q-block, kpos-chunk:
    S^T chunk = k_chunk^T q  -> PSUM [128, 2x512]
    attnw = exp(SCALE * S^T) -> SBUF bf16 (ScalarE, PSUM source)
    o^T  += [v_h | 1]^T attnw -> PSUM [65, 512] accumulated over kpos
  normalize columns of o^T by 1/(rowsum+1) (gpsimd partition-broadcast)
  out  = o^T^T W_out + b_out               (bias via K=1 ones matmul)
"""

import sys

sys.path.insert(0, "/opt/trn_rl_repo")

from contextlib import ExitStack

import numpy as np

import concourse.bass as bass
import concourse.tile as tile
from concourse import bacc, mybir

P = 128
B, N, D = 4, 2048, 1024
H, DH = 16, 64
INNER = H * DH  # 1024
SCALE = DH ** -0.5
NQ = N // 2     # queries per core
NCORES = 8

F32 = mybir.dt.float32
BF16 = mybir.dt.bfloat16

DC = D // P         # 8 chunks of the model dim
IC = INNER // P     # 8 chunks of the inner dim
TK = N // P         # 16 kpos chunks
NG = H // 2         # 8 head pairs
QB = NQ // 512      # 2 query blocks of 512

DEBUG_TAPS = False  # add per-stage DRAM dumps (debugging only)


def _build_body(ctx: ExitStack, tc, out_ext, x_ext, wqkv_ext, wout_ext, bout_ext):
    nc = tc.nc

    dram_pool = ctx.enter_context(tc.tile_pool(name="dram", bufs=1, space="DRAM"))
    xbf_dram = dram_pool.tile([N, D], BF16, tag="xbf")

    consts = ctx.enter_context(tc.tile_pool(name="consts", bufs=1))
    xstage = ctx.enter_context(tc.tile_pool(name="xstage", bufs=2))
    xbfst = ctx.enter_context(tc.tile_pool(name="xbfst", bufs=2))
    wstage = ctx.enter_context(tc.tile_pool(name="wstage", bufs=2))
    persist = ctx.enter_context(tc.tile_pool(name="persist", bufs=1))
    attnw_pool = ctx.enter_context(tc.tile_pool(name="attnw", bufs=5))
    bc_pool = ctx.enter_context(tc.tile_pool(name="bcast", bufs=2))
    outst = ctx.enter_context(tc.tile_pool(name="outst", bufs=2))

    psum_proj = ctx.enter_context(tc.tile_pool(name="psum_proj", bufs=2, space="PSUM"))
    psum_st = ctx.enter_context(tc.tile_pool(name="psum_st", bufs=2, space="PSUM"))
    psum_o = ctx.enter_context(tc.tile_pool(name="psum_o", bufs=2, space="PSUM"))

    # ---- persistent SBUF arrays ----
    xT = persist.tile([P, DC, N], BF16, tag="xT")            # [d-in-chunk, c, token]
    qT = persist.tile([P, IC, NQ], BF16, tag="qT")
    kT = persist.tile([P, IC, N], BF16, tag="kT")
    v_sb = persist.tile([P, TK, H, DH + 1], BF16, tag="v")   # col DH is ones
    oT = persist.tile([P, IC, NQ], BF16, tag="oT")


    ones_lhsT = consts.tile([1, P], BF16, tag="ones")
    nc.vector.memset(ones_lhsT, 1.0)
    ones_f32 = consts.tile([1, P], F32, tag="onesf")
    nc.vector.memset(ones_f32, 1.0)
    bout_bf = consts.tile([1, D], BF16, tag="bout")
    bout_f32 = consts.tile([1, D], F32, tag="boutf")
    nc.sync.dma_start(bout_f32, bout_ext)
    nc.vector.tensor_copy(bout_bf, bout_f32)

    # ones column of v (written once; v evictions fill the rest)
    for t in range(TK):
        nc.vector.memset(v_sb[:, t, :, DH:DH + 1], 1.0)

    # ---- x: load, cast to bf16, bounce to DRAM, XBAR-transpose back ----
    for r in range(TK):
        x_f = xstage.tile([P, D], F32, tag="xf")
        nc.sync.dma_start(x_f, x_ext[r * P:(r + 1) * P, :])
        x_b = xbfst.tile([P, D], BF16, tag="xb")
        nc.vector.tensor_copy(x_b, x_f)
        nc.sync.dma_start(xbf_dram[r * P:(r + 1) * P, :], x_b)
        for c in range(DC):
            # transpose 128x128 block: rows r*P.., cols c*P.. -> xT[:, c, r*P..]
            nc.scalar.dma_start_transpose(
                xT[:, c, r * P:(r + 1) * P],
                xbf_dram[r * P:(r + 1) * P, c * P:(c + 1) * P],
            )

    # ---- projections, streaming W_qkv column blocks ----
    wqkv_view = wqkv_ext.rearrange("(c p) f -> p c f", p=P)   # [128, DC, 3072]

    def load_w_block(pool, m, tag):
        """DMA one [1024, 128] column block of W_qkv, cast to bf16."""
        w_f = wstage.tile([P, DC, P], F32, tag="wf", name=f"wf_{tag}_{m}")
        nc.sync.dma_start(w_f, wqkv_view[:, :, m * P:(m + 1) * P])
        w_b = pool.tile([P, DC, P], BF16, tag="wqk", name=f"wb_{tag}_{m}")
        nc.vector.tensor_copy(w_b, w_f)
        return w_b

    with tc.tile_pool(name="wqk_pool", bufs=3) as wqk_pool, \
         tc.tile_pool(name="wv_pool", bufs=1) as wv_pool:
        # qT[:, m, :]: lhsT = W_q block m, rhs = xT[:, c, 0:NQ]
        for m in range(IC):
            w_b = load_w_block(wqk_pool, m, "q")
            ps = [psum_proj.tile([P, 512], F32, tag="pproj", name=f"pq_{m}_{j}")
                  for j in range(QB)]
            for c in range(DC):
                for j in range(QB):
                    nc.tensor.matmul(
                        ps[j], w_b[:, c, :],
                        xT[:, c, j * 512:(j + 1) * 512],
                        start=(c == 0), stop=(c == DC - 1),
                    )
            for j in range(QB):
                nc.vector.tensor_copy(qT[:, m, j * 512:(j + 1) * 512], ps[j])

        # kT[:, m, :]: lhsT = W_k block m, rhs = xT (all tokens)
        for m in range(IC):
            w_b = load_w_block(wqk_pool, 8 + m, "k")
            for g2 in range(2):
                ps = [psum_proj.tile([P, 512], F32, tag="pproj",
                                     name=f"pk_{m}_{g2}_{j}") for j in range(2)]
                for c in range(DC):
                    for j in range(2):
                        nj = g2 * 2 + j
                        nc.tensor.matmul(
                            ps[j], w_b[:, c, :],
                            xT[:, c, nj * 512:(nj + 1) * 512],
                            start=(c == 0), stop=(c == DC - 1),
                        )
                for j in range(2):
                    nj = g2 * 2 + j
                    nc.vector.tensor_copy(kT[:, m, nj * 512:(nj + 1) * 512], ps[j])

        # v[t]: lhsT = xT[:, c, t*P..], rhs = W_v half nh (resident, reused)
        for nh in range(2):
            wvh = wv_pool.tile([P, DC, 512], BF16, tag="wvh", name=f"wvh_{nh}")
            for blk in range(4):
                m = 16 + nh * 4 + blk
                w_f = wstage.tile([P, DC, P], F32, tag="wf", name=f"wf_v_{m}")
                nc.sync.dma_start(w_f, wqkv_view[:, :, m * P:(m + 1) * P])
                nc.vector.tensor_copy(wvh[:, :, blk * P:(blk + 1) * P], w_f)
            for t in range(TK):
                ps = psum_proj.tile([P, 512], F32, tag="pproj", name=f"pv_{t}_{nh}")
                for c in range(DC):
                    nc.tensor.matmul(
                        ps, xT[:, c, t * P:(t + 1) * P], wvh[:, c, :],
                        start=(c == 0), stop=(c == DC - 1),
                    )
                nc.vector.tensor_copy(
                    v_sb[:, t, nh * 8:(nh + 1) * 8, 0:DH],
                    ps.rearrange("p (h d) -> p h d", h=8),
                )

    # ---- attention ----
    # head pair g = heads (2g, 2g+1); rows of kT/qT chunk g: [0:64] and [64:128]
    for g in range(NG):
        for qb in range(QB):
            o_ps = [psum_o.tile([P, 512], F32, tag="opsum", name=f"po_{g}_{qb}_{j}") for j in range(2)]
            for kc in range(TK):
                st = psum_st.tile([P, 1024], F32, tag="st")
                for h01 in range(2):
                    lo = h01 * 64
                    nc.tensor.matmul(
                        st[:, h01 * 512:(h01 + 1) * 512],
                        kT[lo:lo + 64, g, kc * P:(kc + 1) * P],
                        qT[lo:lo + 64, g, qb * 512:(qb + 1) * 512],
                        start=True, stop=True,
                    )
                aw = attnw_pool.tile([P, 1024], BF16, tag="aw")
                nc.scalar.activation(
                    aw, st, mybir.ActivationFunctionType.Exp, scale=SCALE,
                )
                for h01 in range(2):
                    h = 2 * g + h01
                    nc.tensor.matmul(
                        o_ps[h01][0:DH + 1, :],
                        v_sb[:, kc, h, :],
                        aw[:, h01 * 512:(h01 + 1) * 512],
                        start=(kc == 0), stop=(kc == TK - 1),
                    )
            # normalize & evict: oT rows = o_unnorm * 1/(rowsum + 1);
            # 1/(D) broadcast across partitions via K=1 outer product on PE
            bc = psum_proj.tile([P, 512], F32, tag="pproj", name=f"bc_{g}_{qb}")
            for h01 in range(2):
                d_sb = bc_pool.tile([1, 512], F32, tag="dsb",
                                    name=f"d_{g}_{qb}_{h01}")
                nc.vector.tensor_scalar_add(d_sb, o_ps[h01][DH:DH + 1, :], 1.0)
                nc.vector.reciprocal(d_sb, d_sb)
                nc.tensor.matmul(
                    bc[h01 * 64:(h01 + 1) * 64, :], ones_f32[0:1, 0:64], d_sb,
                    start=True, stop=True, tile_position=(0, h01 * 64),
                )
                nc.vector.tensor_copy(
                    oT[h01 * 64:(h01 + 1) * 64, g, qb * 512:(qb + 1) * 512],
                    o_ps[h01][0:DH, :],
                )
            nc.vector.tensor_mul(
                oT[:, g, qb * 512:(qb + 1) * 512],
                oT[:, g, qb * 512:(qb + 1) * 512],
                bc,
            )

    # ---- W_out load (late: reuses the closed W_qkv pool space) ----
    wout = persist.tile([P, IC, D], BF16, tag="wout")
    wout_view = wout_ext.rearrange("(c p) f -> p c f", p=P)   # [128, IC, 1024]
    for m in range(D // P):
        w_f = wstage.tile([P, IC, P], F32, tag="wf", name=f"wf_o_{m}")
        nc.sync.dma_start(w_f, wout_view[:, :, m * P:(m + 1) * P])
        nc.vector.tensor_copy(wout[:, :, m * P:(m + 1) * P], w_f)

    if DEBUG_TAPS:
        for nm, tl in [("dbg_xT", xT), ("dbg_qT", qT), ("dbg_kT", kT),
                       ("dbg_v", v_sb), ("dbg_oT", oT)]:
            dbg = nc.dram_tensor(nm, list(tl.shape), BF16,
                                 kind="ExternalOutput").ap()
            nc.sync.dma_start(dbg, tl[:])

    # ---- output projection + bias ----
    for t in range(NQ // P):          # 8 row chunks of the output
        for fh in range(2):           # two 512-wide column halves
            ps = psum_proj.tile([P, 512], F32, tag="pproj", name=f"pout_{t}_{fh}")
            for c in range(IC):
                nc.tensor.matmul(
                    ps, oT[:, c, t * P:(t + 1) * P],
                    wout[:, c, fh * 512:(fh + 1) * 512],
                    start=(c == 0), stop=False,
                )
            nc.tensor.matmul(
                ps, ones_lhsT, bout_bf[:, fh * 512:(fh + 1) * 512],
                start=False, stop=True,
            )
            o_sb = outst.tile([P, 512], F32, tag="osb")
            nc.vector.tensor_copy(o_sb, ps)
            nc.sync.dma_start(
                out_ext[t * P:(t + 1) * P, fh * 512:(fh + 1) * 512], o_sb,
            )


def build():
    nc = bacc.Bacc("TRN2", target_bir_lowering=False, debug=False,
                   num_devices=NCORES)
    x_ext = nc.dram_tensor("x", [N, D], F32, kind="ExternalInput").ap()
    wqkv_ext = nc.dram_tensor("w_qkv", [D, 3 * INNER], F32, kind="ExternalInput").ap()
    wout_ext = nc.dram_tensor("w_out", [INNER, D], F32, kind="ExternalInput").ap()
    bout_ext = nc.dram_tensor("b_out", [1, D], F32, kind="ExternalInput").ap()
    out_ext = nc.dram_tensor("out", [NQ, D], F32, kind="ExternalOutput").ap()

    with tile.TileContext(nc) as tc:
        with ExitStack() as ctx:
            _build_body(ctx, tc, out_ext, x_ext, wqkv_ext, wout_ext, bout_ext)
    nc.compile()
    return nc


_NC_CACHE = None


def _get_nc():
    global _NC_CACHE
    if _NC_CACHE is None:
        _NC_CACHE = build()
    return _NC_CACHE


def make_in_maps(x, W_qkv, W_out, b_out):
    x = np.ascontiguousarray(np.asarray(x, dtype=np.float32))
    W_qkv = np.ascontiguousarray(np.asarray(W_qkv, dtype=np.float32))
    W_out = np.ascontiguousarray(np.asarray(W_out, dtype=np.float32))
    b_out = np.ascontiguousarray(np.asarray(b_out, dtype=np.float32)).reshape(1, D)
    in_maps = []
    for core in range(NCORES):
        bi, s = core // 2, core % 2
        xb = x[bi]
        if s == 1:  # rotate so this core's queries are rows 0:NQ
            xb = np.concatenate([xb[NQ:], xb[:NQ]], axis=0)
        in_maps.append({
            "x": np.ascontiguousarray(xb),
            "w_qkv": W_qkv,
            "w_out": W_out,
            "b_out": b_out,
        })
    return in_maps


def assemble(outs):
    full = np.empty((B, N, D), np.float32)
    for core in range(NCORES):
        bi, s = core // 2, core % 2
        full[bi, s * NQ:(s + 1) * NQ] = outs[core]
    return full


def kernel(x, W_qkv, W_out, b_out):
    from concourse.bass_utils import run_bass_kernel_spmd

    nc = _get_nc()
    in_maps = make_in_maps(x, W_qkv, W_out, b_out)
    res = run_bass_kernel_spmd(nc, in_maps, core_ids=list(range(NCORES)))
    return assemble([r["out"] for r in res.results])


# revision 27
# speedup vs baseline: 1.3827x; 1.3827x over previous
"""Distributed multi-head attention kernel for 8 Trainium2 NeuronCores.

Problem: x[4,2048,1024] -> qkv proj -> 16-head attention (add_zero_attn)
         -> out proj + bias -> [4,2048,1024]

Sharding: 8 cores = 4 batches x 2 query-halves. Each core computes the
full K/V for its batch (KV projection duplicated across the pair, ~4GFLOP,
far cheaper than any 2-rank collective on this fabric) and attention +
output projection for its own 1024 queries. Zero collectives; host
reassembles by concatenation only.

add_zero_attn appends a zero key & value token: the value row is zero so it
only adds +1 to each softmax denominator. We therefore never materialize it;
denominators come from a ones-column appended to V (column 64 of each head's
v tile) and get +1 before the reciprocal.

Math per core (all matmuls in bf16, accumulation f32):
  xT   = transpose(x)                      via XBAR DMA-transpose (bf16)
  qT   = W_q^T x_q^T   [1024, 1024]        (inner on partitions)
  kT   = W_k^T x^T     [1024, 2048]
  v    = x W_v         [2048, 1024(+ones)] (kpos on partitions)
  per head pair, q-block, kpos-chunk:
    S^T chunk = k_chunk^T q  -> PSUM [128, 2x512]
    attnw = exp(SCALE * S^T) -> SBUF bf16 (ScalarE, PSUM source)
    o^T  += [v_h | 1]^T attnw -> PSUM [65, 512] accumulated over kpos
  normalize columns of o^T by 1/(rowsum+1) (gpsimd partition-broadcast)
  out  = o^T^T W_out + b_out               (bias via K=1 ones matmul)
"""

import sys

sys.path.insert(0, "/opt/trn_rl_repo")

from contextlib import ExitStack

import numpy as np

import concourse.bass as bass
import concourse.tile as tile
from concourse import bacc, mybir

P = 128
B, N, D = 4, 2048, 1024
H, DH = 16, 64
INNER = H * DH  # 1024
SCALE = DH ** -0.5
NQ = N // 2     # queries per core
NCORES = 8

F32 = mybir.dt.float32
BF16 = mybir.dt.bfloat16

DC = D // P         # 8 chunks of the model dim
IC = INNER // P     # 8 chunks of the inner dim
TK = N // P         # 16 kpos chunks
NG = H // 2         # 8 head pairs
QB = NQ // 512      # 2 query blocks of 512

DEBUG_TAPS = False  # add per-stage DRAM dumps (debugging only)


def _build_body(ctx: ExitStack, tc, out_ext, x_ext, wqkv_ext, wout_ext, bout_ext):
    nc = tc.nc

    dram_pool = ctx.enter_context(tc.tile_pool(name="dram", bufs=1, space="DRAM"))
    xbf_dram = dram_pool.tile([N, D], BF16, tag="xbf")

    consts = ctx.enter_context(tc.tile_pool(name="consts", bufs=1))
    xstage = ctx.enter_context(tc.tile_pool(name="xstage", bufs=2))
    xbfst = ctx.enter_context(tc.tile_pool(name="xbfst", bufs=2))
    wstage = ctx.enter_context(tc.tile_pool(name="wstage", bufs=2))
    persist = ctx.enter_context(tc.tile_pool(name="persist", bufs=1))
    attnw_pool = ctx.enter_context(tc.tile_pool(name="attnw", bufs=5))
    bc_pool = ctx.enter_context(tc.tile_pool(name="bcast", bufs=2))
    outst = ctx.enter_context(tc.tile_pool(name="outst", bufs=2))

    psum_proj = ctx.enter_context(tc.tile_pool(name="psum_proj", bufs=2, space="PSUM"))
    psum_st = ctx.enter_context(tc.tile_pool(name="psum_st", bufs=2, space="PSUM"))
    psum_o = ctx.enter_context(tc.tile_pool(name="psum_o", bufs=2, space="PSUM"))

    # ---- persistent SBUF arrays ----
    xT = persist.tile([P, DC, N], BF16, tag="xT")            # [d-in-chunk, c, token]
    qT = persist.tile([P, IC, NQ], BF16, tag="qT")
    kT = persist.tile([P, IC, N], BF16, tag="kT")
    v_sb = persist.tile([P, TK, H, DH + 1], BF16, tag="v")   # col DH is ones
    oT = persist.tile([P, IC, NQ], BF16, tag="oT")


    ones_lhsT = consts.tile([1, P], BF16, tag="ones")
    nc.vector.memset(ones_lhsT, 1.0)
    ones_f32 = consts.tile([1, P], F32, tag="onesf")
    nc.vector.memset(ones_f32, 1.0)
    bout_bf = consts.tile([1, D], BF16, tag="bout")
    bout_f32 = consts.tile([1, D], F32, tag="boutf")
    nc.gpsimd.dma_start(bout_f32, bout_ext)
    nc.vector.tensor_copy(bout_bf, bout_f32)

    # ones column of v (written once; v evictions fill the rest)
    for t in range(TK):
        nc.vector.memset(v_sb[:, t, :, DH:DH + 1], 1.0)

    # ---- x: load, cast to bf16, bounce to DRAM, XBAR-transpose back ----
    # Transposes ride the Sync HWDGE queue exclusively (no xbar-mode
    # thrash); bulk loads/stores go through gpsimd SWDGE.
    for r in range(TK):
        x_f = xstage.tile([P, D], F32, tag="xf")
        nc.gpsimd.dma_start(x_f, x_ext[r * P:(r + 1) * P, :])
        x_b = xbfst.tile([P, D], BF16, tag="xb")
        nc.vector.tensor_copy(x_b, x_f)
        nc.gpsimd.dma_start(xbf_dram[r * P:(r + 1) * P, :], x_b)
        if r % 8 == 7:
            # transpose a [1024, 128] column slab for each d-chunk
            for c in range(DC):
                nc.sync.dma_start_transpose(
                    xT[:, c, (r - 7) * P:(r + 1) * P],
                    xbf_dram[(r - 7) * P:(r + 1) * P, c * P:(c + 1) * P],
                )

    # ---- projections, streaming W_qkv column blocks ----
    wqkv_view = wqkv_ext.rearrange("(c p) f -> p c f", p=P)   # [128, DC, 3072]

    def load_w_block(pool, m, tag):
        """DMA one [1024, 128] column block of W_qkv, cast to bf16."""
        w_f = wstage.tile([P, DC, P], F32, tag="wf", name=f"wf_{tag}_{m}")
        nc.gpsimd.dma_start(w_f, wqkv_view[:, :, m * P:(m + 1) * P])
        w_b = pool.tile([P, DC, P], BF16, tag="wqk", name=f"wb_{tag}_{m}")
        nc.vector.tensor_copy(w_b, w_f)
        return w_b

    with tc.tile_pool(name="wqk_pool", bufs=3) as wqk_pool, \
         tc.tile_pool(name="wv_pool", bufs=1) as wv_pool:
        # qT[:, m, :]: lhsT = W_q block m, rhs = xT[:, c, 0:NQ]
        for m in range(IC):
            w_b = load_w_block(wqk_pool, m, "q")
            ps = [psum_proj.tile([P, 512], F32, tag="pproj", name=f"pq_{m}_{j}")
                  for j in range(QB)]
            for c in range(DC):
                for j in range(QB):
                    nc.tensor.matmul(
                        ps[j], w_b[:, c, :],
                        xT[:, c, j * 512:(j + 1) * 512],
                        start=(c == 0), stop=(c == DC - 1),
                    )
            for j in range(QB):
                nc.vector.tensor_copy(qT[:, m, j * 512:(j + 1) * 512], ps[j])

        # kT[:, m, :]: lhsT = W_k block m, rhs = xT (all tokens)
        for m in range(IC):
            w_b = load_w_block(wqk_pool, 8 + m, "k")
            for g2 in range(2):
                ps = [psum_proj.tile([P, 512], F32, tag="pproj",
                                     name=f"pk_{m}_{g2}_{j}") for j in range(2)]
                for c in range(DC):
                    for j in range(2):
                        nj = g2 * 2 + j
                        nc.tensor.matmul(
                            ps[j], w_b[:, c, :],
                            xT[:, c, nj * 512:(nj + 1) * 512],
                            start=(c == 0), stop=(c == DC - 1),
                        )
                for j in range(2):
                    nj = g2 * 2 + j
                    nc.vector.tensor_copy(kT[:, m, nj * 512:(nj + 1) * 512], ps[j])

        # v[t]: lhsT = xT[:, c, t*P..], rhs = W_v half nh (resident, reused)
        for nh in range(2):
            wvh = wv_pool.tile([P, DC, 512], BF16, tag="wvh", name=f"wvh_{nh}")
            for blk in range(4):
                m = 16 + nh * 4 + blk
                w_f = wstage.tile([P, DC, P], F32, tag="wf", name=f"wf_v_{m}")
                nc.gpsimd.dma_start(w_f, wqkv_view[:, :, m * P:(m + 1) * P])
                nc.vector.tensor_copy(wvh[:, :, blk * P:(blk + 1) * P], w_f)
            for t in range(TK):
                ps = psum_proj.tile([P, 512], F32, tag="pproj", name=f"pv_{t}_{nh}")
                for c in range(DC):
                    nc.tensor.matmul(
                        ps, xT[:, c, t * P:(t + 1) * P], wvh[:, c, :],
                        start=(c == 0), stop=(c == DC - 1),
                    )
                nc.vector.tensor_copy(
                    v_sb[:, t, nh * 8:(nh + 1) * 8, 0:DH],
                    ps.rearrange("p (h d) -> p h d", h=8),
                )

    # ---- attention ----
    # head pair g = heads (2g, 2g+1); rows of kT/qT chunk g: [0:64] and [64:128]
    for g in range(NG):
        for qb in range(QB):
            o_ps = [psum_o.tile([P, 512], F32, tag="opsum", name=f"po_{g}_{qb}_{j}") for j in range(2)]
            for kc in range(TK):
                st = psum_st.tile([P, 1024], F32, tag="st")
                for h01 in range(2):
                    lo = h01 * 64
                    nc.tensor.matmul(
                        st[:, h01 * 512:(h01 + 1) * 512],
                        kT[lo:lo + 64, g, kc * P:(kc + 1) * P],
                        qT[lo:lo + 64, g, qb * 512:(qb + 1) * 512],
                        start=True, stop=True,
                    )
                aw = attnw_pool.tile([P, 1024], BF16, tag="aw")
                nc.scalar.activation(
                    aw, st, mybir.ActivationFunctionType.Exp, scale=SCALE,
                )
                for h01 in range(2):
                    h = 2 * g + h01
                    nc.tensor.matmul(
                        o_ps[h01][0:DH + 1, :],
                        v_sb[:, kc, h, :],
                        aw[:, h01 * 512:(h01 + 1) * 512],
                        start=(kc == 0), stop=(kc == TK - 1),
                    )
            # normalize & evict: oT rows = o_unnorm * 1/(rowsum + 1);
            # 1/(D) broadcast across partitions via K=1 outer product on PE
            bc = psum_proj.tile([P, 512], F32, tag="pproj", name=f"bc_{g}_{qb}")
            for h01 in range(2):
                d_sb = bc_pool.tile([1, 512], F32, tag="dsb",
                                    name=f"d_{g}_{qb}_{h01}")
                # D + 1 (the zero-attn token) folded into the gather copy
                nc.vector.tensor_scalar_add(d_sb, o_ps[h01][DH:DH + 1, :], 1.0)
                nc.tensor.matmul(
                    bc[h01 * 64:(h01 + 1) * 64, :], ones_f32[0:1, 0:64], d_sb,
                    start=True, stop=True, tile_position=(0, h01 * 64),
                )
                nc.vector.tensor_copy(
                    oT[h01 * 64:(h01 + 1) * 64, g, qb * 512:(qb + 1) * 512],
                    o_ps[h01][0:DH, :],
                )
            # reciprocal on the broadcast tile: full 128 lanes, off the
            # o-psum release path
            nc.vector.reciprocal(bc, bc)
            nc.vector.tensor_mul(
                oT[:, g, qb * 512:(qb + 1) * 512],
                oT[:, g, qb * 512:(qb + 1) * 512],
                bc,
            )

    # ---- W_out load (late: reuses the closed W_qkv pool space) ----
    wout = persist.tile([P, IC, D], BF16, tag="wout")
    wout_view = wout_ext.rearrange("(c p) f -> p c f", p=P)   # [128, IC, 1024]
    for m in range(D // P):
        w_f = wstage.tile([P, IC, P], F32, tag="wf", name=f"wf_o_{m}")
        nc.gpsimd.dma_start(w_f, wout_view[:, :, m * P:(m + 1) * P])
        nc.vector.tensor_copy(wout[:, :, m * P:(m + 1) * P], w_f)

    if DEBUG_TAPS:
        for nm, tl in [("dbg_xT", xT), ("dbg_qT", qT), ("dbg_kT", kT),
                       ("dbg_v", v_sb), ("dbg_oT", oT)]:
            dbg = nc.dram_tensor(nm, list(tl.shape), BF16,
                                 kind="ExternalOutput").ap()
            nc.sync.dma_start(dbg, tl[:])

    # ---- output projection + bias ----
    for t in range(NQ // P):          # 8 row chunks of the output
        for fh in range(2):           # two 512-wide column halves
            ps = psum_proj.tile([P, 512], F32, tag="pproj", name=f"pout_{t}_{fh}")
            for c in range(IC):
                nc.tensor.matmul(
                    ps, oT[:, c, t * P:(t + 1) * P],
                    wout[:, c, fh * 512:(fh + 1) * 512],
                    start=(c == 0), stop=False,
                )
            nc.tensor.matmul(
                ps, ones_lhsT, bout_bf[:, fh * 512:(fh + 1) * 512],
                start=False, stop=True,
            )
            o_sb = outst.tile([P, 512], F32, tag="osb")
            nc.vector.tensor_copy(o_sb, ps)
            nc.gpsimd.dma_start(
                out_ext[t * P:(t + 1) * P, fh * 512:(fh + 1) * 512], o_sb,
            )


def build():
    nc = bacc.Bacc("TRN2", target_bir_lowering=False, debug=False,
                   num_devices=NCORES)
    x_ext = nc.dram_tensor("x", [N, D], F32, kind="ExternalInput").ap()
    wqkv_ext = nc.dram_tensor("w_qkv", [D, 3 * INNER], F32, kind="ExternalInput").ap()
    wout_ext = nc.dram_tensor("w_out", [INNER, D], F32, kind="ExternalInput").ap()
    bout_ext = nc.dram_tensor("b_out", [1, D], F32, kind="ExternalInput").ap()
    out_ext = nc.dram_tensor("out", [NQ, D], F32, kind="ExternalOutput").ap()

    with tile.TileContext(nc) as tc:
        with ExitStack() as ctx:
            _build_body(ctx, tc, out_ext, x_ext, wqkv_ext, wout_ext, bout_ext)
    nc.compile()
    return nc


_NC_CACHE = None


def _get_nc():
    global _NC_CACHE
    if _NC_CACHE is None:
        _NC_CACHE = build()
    return _NC_CACHE


def make_in_maps(x, W_qkv, W_out, b_out):
    x = np.ascontiguousarray(np.asarray(x, dtype=np.float32))
    W_qkv = np.ascontiguousarray(np.asarray(W_qkv, dtype=np.float32))
    W_out = np.ascontiguousarray(np.asarray(W_out, dtype=np.float32))
    b_out = np.ascontiguousarray(np.asarray(b_out, dtype=np.float32)).reshape(1, D)
    in_maps = []
    for core in range(NCORES):
        bi, s = core // 2, core % 2
        xb = x[bi]
        if s == 1:  # rotate so this core's queries are rows 0:NQ
            xb = np.concatenate([xb[NQ:], xb[:NQ]], axis=0)
        in_maps.append({
            "x": np.ascontiguousarray(xb),
            "w_qkv": W_qkv,
            "w_out": W_out,
            "b_out": b_out,
        })
    return in_maps


def assemble(outs):
    full = np.empty((B, N, D), np.float32)
    for core in range(NCORES):
        bi, s = core // 2, core % 2
        full[bi, s * NQ:(s + 1) * NQ] = outs[core]
    return full


def kernel(x, W_qkv, W_out, b_out):
    from concourse.bass_utils import run_bass_kernel_spmd

    nc = _get_nc()
    in_maps = make_in_maps(x, W_qkv, W_out, b_out)
    res = run_bass_kernel_spmd(nc, in_maps, core_ids=list(range(NCORES)))
    return assemble([r["out"] for r in res.results])


# revision 28
# speedup vs baseline: 1.3981x; 1.0111x over previous
"""Distributed multi-head attention kernel for 8 Trainium2 NeuronCores.

Problem: x[4,2048,1024] -> qkv proj -> 16-head attention (add_zero_attn)
         -> out proj + bias -> [4,2048,1024]

Sharding: 8 cores = 4 batches x 2 query-halves. Each core computes the
full K/V for its batch (KV projection duplicated across the pair, ~4GFLOP,
far cheaper than any 2-rank collective on this fabric) and attention +
output projection for its own 1024 queries. Zero collectives; host
reassembles by concatenation only.

add_zero_attn appends a zero key & value token: the value row is zero so it
only adds +1 to each softmax denominator. We therefore never materialize it;
denominators come from a ones-column appended to V (column 64 of each head's
v tile) and get +1 before the reciprocal.

Math per core (all matmuls in bf16, accumulation f32):
  xT   = transpose(x)                      via XBAR DMA-transpose (bf16)
  qT   = W_q^T x_q^T   [1024, 1024]        (inner on partitions)
  kT   = W_k^T x^T     [1024, 2048]
  v    = x W_v         [2048, 1024(+ones)] (kpos on partitions)
  per head pair, q-block, kpos-chunk:
    S^T chunk = k_chunk^T q  -> PSUM [128, 2x512]
    attnw = exp(SCALE * S^T) -> SBUF bf16 (ScalarE, PSUM source)
    o^T  += [v_h | 1]^T attnw -> PSUM [65, 512] accumulated over kpos
  normalize columns of o^T by 1/(rowsum+1) (gpsimd partition-broadcast)
  out  = o^T^T W_out + b_out               (bias via K=1 ones matmul)
"""

import sys

sys.path.insert(0, "/opt/trn_rl_repo")

from contextlib import ExitStack

import numpy as np

import concourse.bass as bass
import concourse.tile as tile
from concourse import bacc, mybir

P = 128
B, N, D = 4, 2048, 1024
H, DH = 16, 64
INNER = H * DH  # 1024
SCALE = DH ** -0.5
NQ = N // 2     # queries per core
NCORES = 8

F32 = mybir.dt.float32
BF16 = mybir.dt.bfloat16

DC = D // P         # 8 chunks of the model dim
IC = INNER // P     # 8 chunks of the inner dim
TK = N // P         # 16 kpos chunks
NG = H // 2         # 8 head pairs
QB = NQ // 512      # 2 query blocks of 512

DEBUG_TAPS = False  # add per-stage DRAM dumps (debugging only)


def _build_body(ctx: ExitStack, tc, out_ext, x_ext, wqkv_ext, wout_ext, bout_ext):
    nc = tc.nc

    dram_pool = ctx.enter_context(tc.tile_pool(name="dram", bufs=1, space="DRAM"))
    xbf_dram = dram_pool.tile([N, D], BF16, tag="xbf")

    consts = ctx.enter_context(tc.tile_pool(name="consts", bufs=1))
    xstage = ctx.enter_context(tc.tile_pool(name="xstage", bufs=2))
    xbfst = ctx.enter_context(tc.tile_pool(name="xbfst", bufs=2))
    wstage = ctx.enter_context(tc.tile_pool(name="wstage", bufs=2))
    persist = ctx.enter_context(tc.tile_pool(name="persist", bufs=1))
    attnw_pool = ctx.enter_context(tc.tile_pool(name="attnw", bufs=5))
    bc_pool = ctx.enter_context(tc.tile_pool(name="bcast", bufs=2))
    outst = ctx.enter_context(tc.tile_pool(name="outst", bufs=2))

    psum_univ = ctx.enter_context(tc.tile_pool(name="psum_univ", bufs=4, space="PSUM"))
    psum_st = ctx.enter_context(tc.tile_pool(name="psum_st", bufs=2, space="PSUM"))

    # ---- persistent SBUF arrays ----
    xT = persist.tile([P, DC, N], BF16, tag="xT")            # [d-in-chunk, c, token]
    qT = persist.tile([P, IC, NQ], BF16, tag="qT")
    kT = persist.tile([P, IC, N], BF16, tag="kT")
    v_sb = persist.tile([P, TK, H, DH + 1], BF16, tag="v")   # col DH is ones
    oT = persist.tile([P, IC, NQ], BF16, tag="oT")


    ones_lhsT = consts.tile([1, P], BF16, tag="ones")
    nc.vector.memset(ones_lhsT, 1.0)
    ones_f32 = consts.tile([1, P], F32, tag="onesf")
    nc.vector.memset(ones_f32, 1.0)
    bout_bf = consts.tile([1, D], BF16, tag="bout")
    bout_f32 = consts.tile([1, D], F32, tag="boutf")
    nc.gpsimd.dma_start(bout_f32, bout_ext)
    nc.vector.tensor_copy(bout_bf, bout_f32)

    # ones column of v (written once; v evictions fill the rest)
    for t in range(TK):
        nc.vector.memset(v_sb[:, t, :, DH:DH + 1], 1.0)

    # ---- x: load, cast to bf16, bounce to DRAM, XBAR-transpose back ----
    # Transposes ride the Sync HWDGE queue exclusively (no xbar-mode
    # thrash); bulk loads/stores go through gpsimd SWDGE.
    for r in range(TK):
        x_f = xstage.tile([P, D], F32, tag="xf")
        nc.gpsimd.dma_start(x_f, x_ext[r * P:(r + 1) * P, :])
        x_b = xbfst.tile([P, D], BF16, tag="xb")
        nc.vector.tensor_copy(x_b, x_f)
        nc.gpsimd.dma_start(xbf_dram[r * P:(r + 1) * P, :], x_b)
        if r % 4 == 3:
            # transpose a [512, 128] column slab for each d-chunk
            for c in range(DC):
                nc.sync.dma_start_transpose(
                    xT[:, c, (r - 3) * P:(r + 1) * P],
                    xbf_dram[(r - 3) * P:(r + 1) * P, c * P:(c + 1) * P],
                )

    # ---- projections, streaming W_qkv column blocks ----
    wqkv_view = wqkv_ext.rearrange("(c p) f -> p c f", p=P)   # [128, DC, 3072]

    def load_w_block(pool, m, tag):
        """DMA one [1024, 128] column block of W_qkv, cast to bf16."""
        w_f = wstage.tile([P, DC, P], F32, tag="wf", name=f"wf_{tag}_{m}")
        nc.gpsimd.dma_start(w_f, wqkv_view[:, :, m * P:(m + 1) * P])
        w_b = pool.tile([P, DC, P], BF16, tag="wqk", name=f"wb_{tag}_{m}")
        nc.vector.tensor_copy(w_b, w_f)
        return w_b

    with tc.tile_pool(name="wqk_pool", bufs=3) as wqk_pool, \
         tc.tile_pool(name="wv_pool", bufs=1) as wv_pool:
        # qT[:, m, :]: lhsT = W_q block m, rhs = xT[:, c, 0:NQ]
        for m in range(IC):
            w_b = load_w_block(wqk_pool, m, "q")
            ps = [psum_univ.tile([P, 512], F32, tag="u512", name=f"pq_{m}_{j}")
                  for j in range(QB)]
            for c in range(DC):
                for j in range(QB):
                    nc.tensor.matmul(
                        ps[j], w_b[:, c, :],
                        xT[:, c, j * 512:(j + 1) * 512],
                        start=(c == 0), stop=(c == DC - 1),
                    )
            for j in range(QB):
                nc.vector.tensor_copy(qT[:, m, j * 512:(j + 1) * 512], ps[j])

        # kT[:, m, :]: lhsT = W_k block m, rhs = xT (all tokens)
        for m in range(IC):
            w_b = load_w_block(wqk_pool, 8 + m, "k")
            for g2 in range(2):
                ps = [psum_univ.tile([P, 512], F32, tag="u512",
                                     name=f"pk_{m}_{g2}_{j}") for j in range(2)]
                for c in range(DC):
                    for j in range(2):
                        nj = g2 * 2 + j
                        nc.tensor.matmul(
                            ps[j], w_b[:, c, :],
                            xT[:, c, nj * 512:(nj + 1) * 512],
                            start=(c == 0), stop=(c == DC - 1),
                        )
                for j in range(2):
                    nj = g2 * 2 + j
                    nc.vector.tensor_copy(kT[:, m, nj * 512:(nj + 1) * 512], ps[j])

        # v[t]: lhsT = xT[:, c, t*P..], rhs = W_v half nh (resident, reused)
        for nh in range(2):
            wvh = wv_pool.tile([P, DC, 512], BF16, tag="wvh", name=f"wvh_{nh}")
            for blk in range(4):
                m = 16 + nh * 4 + blk
                w_f = wstage.tile([P, DC, P], F32, tag="wf", name=f"wf_v_{m}")
                nc.gpsimd.dma_start(w_f, wqkv_view[:, :, m * P:(m + 1) * P])
                nc.vector.tensor_copy(wvh[:, :, blk * P:(blk + 1) * P], w_f)
            for t in range(TK):
                ps = psum_univ.tile([P, 512], F32, tag="u512", name=f"pv_{t}_{nh}")
                for c in range(DC):
                    nc.tensor.matmul(
                        ps, xT[:, c, t * P:(t + 1) * P], wvh[:, c, :],
                        start=(c == 0), stop=(c == DC - 1),
                    )
                nc.vector.tensor_copy(
                    v_sb[:, t, nh * 8:(nh + 1) * 8, 0:DH],
                    ps.rearrange("p (h d) -> p h d", h=8),
                )

    # ---- attention ----
    # head pair g = heads (2g, 2g+1); rows of kT/qT chunk g: [0:64] and [64:128]
    for g in range(NG):
        for qb in range(QB):
            o_ps = [psum_univ.tile([P, 512], F32, tag="u512", name=f"po_{g}_{qb}_{j}") for j in range(2)]
            for kc in range(TK):
                st = psum_st.tile([P, 1024], F32, tag="st")
                for h01 in range(2):
                    lo = h01 * 64
                    nc.tensor.matmul(
                        st[:, h01 * 512:(h01 + 1) * 512],
                        kT[lo:lo + 64, g, kc * P:(kc + 1) * P],
                        qT[lo:lo + 64, g, qb * 512:(qb + 1) * 512],
                        start=True, stop=True,
                    )
                aw = attnw_pool.tile([P, 1024], BF16, tag="aw")
                nc.scalar.activation(
                    aw, st, mybir.ActivationFunctionType.Exp, scale=SCALE,
                )
                for h01 in range(2):
                    h = 2 * g + h01
                    nc.tensor.matmul(
                        o_ps[h01][0:DH + 1, :],
                        v_sb[:, kc, h, :],
                        aw[:, h01 * 512:(h01 + 1) * 512],
                        start=(kc == 0), stop=(kc == TK - 1),
                    )
            # normalize & evict: oT rows = o_unnorm * 1/(rowsum + 1);
            # 1/(D) broadcast across partitions via K=1 outer product on PE
            bc = psum_univ.tile([P, 512], F32, tag="u512", name=f"bc_{g}_{qb}")
            for h01 in range(2):
                d_sb = bc_pool.tile([1, 512], F32, tag="dsb",
                                    name=f"d_{g}_{qb}_{h01}")
                # D + 1 (the zero-attn token) folded into the gather copy
                nc.vector.tensor_scalar_add(d_sb, o_ps[h01][DH:DH + 1, :], 1.0)
                nc.tensor.matmul(
                    bc[h01 * 64:(h01 + 1) * 64, :], ones_f32[0:1, 0:64], d_sb,
                    start=True, stop=True, tile_position=(0, h01 * 64),
                )
                nc.vector.tensor_copy(
                    oT[h01 * 64:(h01 + 1) * 64, g, qb * 512:(qb + 1) * 512],
                    o_ps[h01][0:DH, :],
                )
            # reciprocal on the broadcast tile: full 128 lanes, off the
            # o-psum release path
            nc.vector.reciprocal(bc, bc)
            nc.vector.tensor_mul(
                oT[:, g, qb * 512:(qb + 1) * 512],
                oT[:, g, qb * 512:(qb + 1) * 512],
                bc,
            )

    # ---- W_out load (late: reuses the closed W_qkv pool space) ----
    wout = persist.tile([P, IC, D], BF16, tag="wout")
    wout_view = wout_ext.rearrange("(c p) f -> p c f", p=P)   # [128, IC, 1024]
    for m in range(D // P):
        w_f = wstage.tile([P, IC, P], F32, tag="wf", name=f"wf_o_{m}")
        nc.gpsimd.dma_start(w_f, wout_view[:, :, m * P:(m + 1) * P])
        nc.vector.tensor_copy(wout[:, :, m * P:(m + 1) * P], w_f)

    if DEBUG_TAPS:
        for nm, tl in [("dbg_xT", xT), ("dbg_qT", qT), ("dbg_kT", kT),
                       ("dbg_v", v_sb), ("dbg_oT", oT)]:
            dbg = nc.dram_tensor(nm, list(tl.shape), BF16,
                                 kind="ExternalOutput").ap()
            nc.sync.dma_start(dbg, tl[:])

    # ---- output projection + bias ----
    for t in range(NQ // P):          # 8 row chunks of the output
        for fh in range(2):           # two 512-wide column halves
            ps = psum_univ.tile([P, 512], F32, tag="u512", name=f"pout_{t}_{fh}")
            for c in range(IC):
                nc.tensor.matmul(
                    ps, oT[:, c, t * P:(t + 1) * P],
                    wout[:, c, fh * 512:(fh + 1) * 512],
                    start=(c == 0), stop=False,
                )
            nc.tensor.matmul(
                ps, ones_lhsT, bout_bf[:, fh * 512:(fh + 1) * 512],
                start=False, stop=True,
            )
            o_sb = outst.tile([P, 512], F32, tag="osb")
            nc.vector.tensor_copy(o_sb, ps)
            nc.gpsimd.dma_start(
                out_ext[t * P:(t + 1) * P, fh * 512:(fh + 1) * 512], o_sb,
            )


def build():
    nc = bacc.Bacc("TRN2", target_bir_lowering=False, debug=False,
                   num_devices=NCORES)
    x_ext = nc.dram_tensor("x", [N, D], F32, kind="ExternalInput").ap()
    wqkv_ext = nc.dram_tensor("w_qkv", [D, 3 * INNER], F32, kind="ExternalInput").ap()
    wout_ext = nc.dram_tensor("w_out", [INNER, D], F32, kind="ExternalInput").ap()
    bout_ext = nc.dram_tensor("b_out", [1, D], F32, kind="ExternalInput").ap()
    out_ext = nc.dram_tensor("out", [NQ, D], F32, kind="ExternalOutput").ap()

    with tile.TileContext(nc) as tc:
        with ExitStack() as ctx:
            _build_body(ctx, tc, out_ext, x_ext, wqkv_ext, wout_ext, bout_ext)
    nc.compile()
    return nc


_NC_CACHE = None


def _get_nc():
    global _NC_CACHE
    if _NC_CACHE is None:
        _NC_CACHE = build()
    return _NC_CACHE


def make_in_maps(x, W_qkv, W_out, b_out):
    x = np.ascontiguousarray(np.asarray(x, dtype=np.float32))
    W_qkv = np.ascontiguousarray(np.asarray(W_qkv, dtype=np.float32))
    W_out = np.ascontiguousarray(np.asarray(W_out, dtype=np.float32))
    b_out = np.ascontiguousarray(np.asarray(b_out, dtype=np.float32)).reshape(1, D)
    in_maps = []
    for core in range(NCORES):
        bi, s = core // 2, core % 2
        xb = x[bi]
        if s == 1:  # rotate so this core's queries are rows 0:NQ
            xb = np.concatenate([xb[NQ:], xb[:NQ]], axis=0)
        in_maps.append({
            "x": np.ascontiguousarray(xb),
            "w_qkv": W_qkv,
            "w_out": W_out,
            "b_out": b_out,
        })
    return in_maps


def assemble(outs):
    full = np.empty((B, N, D), np.float32)
    for core in range(NCORES):
        bi, s = core // 2, core % 2
        full[bi, s * NQ:(s + 1) * NQ] = outs[core]
    return full


def kernel(x, W_qkv, W_out, b_out):
    from concourse.bass_utils import run_bass_kernel_spmd

    nc = _get_nc()
    in_maps = make_in_maps(x, W_qkv, W_out, b_out)
    res = run_bass_kernel_spmd(nc, in_maps, core_ids=list(range(NCORES)))
    return assemble([r["out"] for r in res.results])


# revision 31
# speedup vs baseline: 1.4491x; 1.0365x over previous
"""Distributed multi-head attention kernel for 8 Trainium2 NeuronCores.

Problem: x[4,2048,1024] -> qkv proj -> 16-head attention (add_zero_attn)
         -> out proj + bias -> [4,2048,1024]

Sharding: 8 cores = 4 batches x 2 query-halves. Each core computes the
full K/V for its batch (KV projection duplicated across the pair, ~4GFLOP,
far cheaper than any 2-rank collective on this fabric) and attention +
output projection for its own 1024 queries. Zero collectives; host
reassembles by concatenation only.

add_zero_attn appends a zero key & value token: the value row is zero so it
only adds +1 to each softmax denominator. We therefore never materialize it;
denominators come from a ones-column appended to V (column 64 of each head's
v tile) and get +1 before the reciprocal.

Math per core (all matmuls in bf16, accumulation f32):
  xT   = transpose(x)                      via XBAR DMA-transpose (bf16)
  qT   = W_q^T x_q^T   [1024, 1024]        (inner on partitions)
  kT   = W_k^T x^T     [1024, 2048]
  v    = x W_v         [2048, 1024(+ones)] (kpos on partitions)
  per head pair, q-block, kpos-chunk:
    S^T chunk = k_chunk^T q  -> PSUM [128, 2x512]
    attnw = exp(SCALE * S^T) -> SBUF bf16 (ScalarE, PSUM source)
    o^T  += [v_h | 1]^T attnw -> PSUM [65, 512] accumulated over kpos
  normalize columns of o^T by 1/(rowsum+1) (gpsimd partition-broadcast)
  out  = o^T^T W_out + b_out               (bias via K=1 ones matmul)
"""

import sys

sys.path.insert(0, "/opt/trn_rl_repo")

from contextlib import ExitStack

import numpy as np

import concourse.bass as bass
import concourse.tile as tile
from concourse import bacc, mybir

P = 128
B, N, D = 4, 2048, 1024
H, DH = 16, 64
INNER = H * DH  # 1024
SCALE = DH ** -0.5
NQ = N // 2     # queries per core
NCORES = 8

F32 = mybir.dt.float32
BF16 = mybir.dt.bfloat16

DC = D // P         # 8 chunks of the model dim
IC = INNER // P     # 8 chunks of the inner dim
TK = N // P         # 16 kpos chunks
NG = H // 2         # 8 head pairs
QB = NQ // 512      # 2 query blocks of 512

DEBUG_TAPS = False  # add per-stage DRAM dumps (debugging only)


def _build_body(ctx: ExitStack, tc, out_ext, x_ext, wqkv_ext, wout_ext, bout_ext):
    nc = tc.nc

    dram_pool = ctx.enter_context(tc.tile_pool(name="dram", bufs=1, space="DRAM"))
    xbf_dram = dram_pool.tile([N, D], BF16, tag="xbf")

    consts = ctx.enter_context(tc.tile_pool(name="consts", bufs=1))
    xstage = ctx.enter_context(tc.tile_pool(name="xstage", bufs=2))
    xbfst = ctx.enter_context(tc.tile_pool(name="xbfst", bufs=2))
    wstage = ctx.enter_context(tc.tile_pool(name="wstage", bufs=2))
    persist = ctx.enter_context(tc.tile_pool(name="persist", bufs=1))
    attnw_pool = ctx.enter_context(tc.tile_pool(name="attnw", bufs=4))
    bc_pool = ctx.enter_context(tc.tile_pool(name="bcast", bufs=4))
    outst = ctx.enter_context(tc.tile_pool(name="outst", bufs=2))

    psum_univ = ctx.enter_context(tc.tile_pool(name="psum_univ", bufs=4, space="PSUM"))
    psum_st = ctx.enter_context(tc.tile_pool(name="psum_st", bufs=2, space="PSUM"))

    # ---- persistent SBUF arrays ----
    xT = persist.tile([P, DC, N], BF16, tag="xT")            # [d-in-chunk, c, token]
    qT = persist.tile([P, IC, NQ], BF16, tag="qT")
    kT = persist.tile([P, IC, N], BF16, tag="kT")
    v_sb = persist.tile([P, TK, H, DH + 1], BF16, tag="v")   # col DH is ones
    oT = persist.tile([P, IC, NQ], BF16, tag="oT")


    ones_lhsT = consts.tile([1, P], BF16, tag="ones")
    nc.vector.memset(ones_lhsT, 1.0)
    ones_f32 = consts.tile([1, P], F32, tag="onesf")
    nc.vector.memset(ones_f32, 1.0)
    bout_bf = consts.tile([1, D], BF16, tag="bout")
    bout_f32 = consts.tile([1, D], F32, tag="boutf")
    nc.gpsimd.dma_start(bout_f32, bout_ext)
    nc.vector.tensor_copy(bout_bf, bout_f32)

    # ones column of v (written once; v evictions fill the rest)
    for t in range(TK):
        nc.vector.memset(v_sb[:, t, :, DH:DH + 1], 1.0)

    # ---- x: load, cast to bf16, bounce to DRAM, XBAR-transpose back ----
    # Transposes ride the Sync HWDGE queue exclusively (no xbar-mode
    # thrash); bulk loads/stores go through gpsimd SWDGE.
    for r in range(TK):
        x_f = xstage.tile([P, D], F32, tag="xf")
        nc.sync.dma_start(x_f, x_ext[r * P:(r + 1) * P, :])
        x_b = xbfst.tile([P, D], BF16, tag="xb")
        nc.vector.tensor_copy(x_b, x_f)
        nc.gpsimd.dma_start(xbf_dram[r * P:(r + 1) * P, :], x_b)
        if r % 4 == 3:
            # transpose a [512, 128] column slab for each d-chunk
            for c in range(DC):
                nc.scalar.dma_start_transpose(
                    xT[:, c, (r - 3) * P:(r + 1) * P],
                    xbf_dram[(r - 3) * P:(r + 1) * P, c * P:(c + 1) * P],
                )

    # ---- projections, streaming W_qkv column blocks ----
    wqkv_view = wqkv_ext.rearrange("(c p) f -> p c f", p=P)   # [128, DC, 3072]

    def load_w_block(pool, m, tag):
        """DMA one [1024, 128] column block of W_qkv, cast to bf16."""
        w_f = wstage.tile([P, DC, P], F32, tag="wf", name=f"wf_{tag}_{m}")
        nc.sync.dma_start(w_f, wqkv_view[:, :, m * P:(m + 1) * P])
        w_b = pool.tile([P, DC, P], BF16, tag="wqk", name=f"wb_{tag}_{m}")
        nc.vector.tensor_copy(w_b, w_f)
        return w_b

    with tc.tile_pool(name="wqk_pool", bufs=3) as wqk_pool, \
         tc.tile_pool(name="wv_pool", bufs=1) as wv_pool:
        # qT[:, m, :]: lhsT = W_q block m, rhs = xT[:, c, 0:NQ]
        for m in range(IC):
            w_b = load_w_block(wqk_pool, m, "q")
            ps = [psum_univ.tile([P, 512], F32, tag="u512", name=f"pq_{m}_{j}")
                  for j in range(QB)]
            for c in range(DC):
                for j in range(QB):
                    nc.tensor.matmul(
                        ps[j], w_b[:, c, :],
                        xT[:, c, j * 512:(j + 1) * 512],
                        start=(c == 0), stop=(c == DC - 1),
                    )
            for j in range(QB):
                nc.vector.tensor_copy(qT[:, m, j * 512:(j + 1) * 512], ps[j])

        # kT[:, m, :]: lhsT = W_k block m, rhs = xT (all tokens)
        for m in range(IC):
            w_b = load_w_block(wqk_pool, 8 + m, "k")
            for g2 in range(2):
                ps = [psum_univ.tile([P, 512], F32, tag="u512",
                                     name=f"pk_{m}_{g2}_{j}") for j in range(2)]
                for c in range(DC):
                    for j in range(2):
                        nj = g2 * 2 + j
                        nc.tensor.matmul(
                            ps[j], w_b[:, c, :],
                            xT[:, c, nj * 512:(nj + 1) * 512],
                            start=(c == 0), stop=(c == DC - 1),
                        )
                for j in range(2):
                    nj = g2 * 2 + j
                    nc.vector.tensor_copy(kT[:, m, nj * 512:(nj + 1) * 512], ps[j])

        # v[t]: lhsT = xT[:, c, t*P..], rhs = W_v half nh (resident, reused)
        for nh in range(2):
            wvh = wv_pool.tile([P, DC, 512], BF16, tag="wvh", name=f"wvh_{nh}")
            for blk in range(4):
                m = 16 + nh * 4 + blk
                w_f = wstage.tile([P, DC, P], F32, tag="wf", name=f"wf_v_{m}")
                nc.sync.dma_start(w_f, wqkv_view[:, :, m * P:(m + 1) * P])
                nc.vector.tensor_copy(wvh[:, :, blk * P:(blk + 1) * P], w_f)
            for t in range(TK):
                ps = psum_univ.tile([P, 512], F32, tag="u512", name=f"pv_{t}_{nh}")
                for c in range(DC):
                    nc.tensor.matmul(
                        ps, xT[:, c, t * P:(t + 1) * P], wvh[:, c, :],
                        start=(c == 0), stop=(c == DC - 1),
                    )
                nc.vector.tensor_copy(
                    v_sb[:, t, nh * 8:(nh + 1) * 8, 0:DH],
                    ps.rearrange("p (h d) -> p h d", h=8),
                )

    # ---- attention ----
    # head pair g = heads (2g, 2g+1); rows of kT/qT chunk g: [0:64] and [64:128]
    pending = []

    def flush_normalize():
        # bc broadcast + reciprocal + in-place normalize for a finished
        # group; deferred so the bc matmul never head-of-line-blocks PE
        if not pending:
            return
        gg, gqb, dsbs = pending.pop(0)
        bc = psum_univ.tile([P, 512], F32, tag="u512", name=f"bc_{gg}_{gqb}")
        for h01 in range(2):
            nc.tensor.matmul(
                bc[h01 * 64:(h01 + 1) * 64, :], ones_f32[0:1, 0:64],
                dsbs[h01], start=True, stop=True, tile_position=(0, h01 * 64),
            )
        nc.vector.reciprocal(bc, bc)
        nc.vector.tensor_mul(
            oT[:, gg, gqb * 512:(gqb + 1) * 512],
            oT[:, gg, gqb * 512:(gqb + 1) * 512],
            bc,
        )

    for g in range(NG):
        for qb in range(QB):
            o_ps = [psum_univ.tile([P, 512], F32, tag="u512", name=f"po_{g}_{qb}_{j}") for j in range(2)]
            for kc in range(TK):
                if kc == 4:
                    flush_normalize()
                st = psum_st.tile([P, 1024], F32, tag="st")
                for h01 in range(2):
                    lo = h01 * 64
                    nc.tensor.matmul(
                        st[:, h01 * 512:(h01 + 1) * 512],
                        kT[lo:lo + 64, g, kc * P:(kc + 1) * P],
                        qT[lo:lo + 64, g, qb * 512:(qb + 1) * 512],
                        start=True, stop=True,
                    )
                aw = attnw_pool.tile([P, 1024], BF16, tag="aw")
                nc.scalar.activation(
                    aw, st, mybir.ActivationFunctionType.Exp, scale=SCALE,
                )
                for h01 in range(2):
                    h = 2 * g + h01
                    nc.tensor.matmul(
                        o_ps[h01][0:DH + 1, :],
                        v_sb[:, kc, h, :],
                        aw[:, h01 * 512:(h01 + 1) * 512],
                        start=(kc == 0), stop=(kc == TK - 1),
                    )
            # evict: D rows (+1 for the zero-attn token) and unnormalized oT
            dsbs = []
            for h01 in range(2):
                d_sb = bc_pool.tile([1, 512], F32, tag="dsb",
                                    name=f"d_{g}_{qb}_{h01}")
                nc.vector.tensor_scalar_add(d_sb, o_ps[h01][DH:DH + 1, :], 1.0)
                dsbs.append(d_sb)
                nc.vector.tensor_copy(
                    oT[h01 * 64:(h01 + 1) * 64, g, qb * 512:(qb + 1) * 512],
                    o_ps[h01][0:DH, :],
                )
            pending.append((g, qb, dsbs))

    while pending:
        flush_normalize()

    # ---- W_out load (late: reuses the closed W_qkv pool space) ----
    wout = persist.tile([P, IC, D], BF16, tag="wout")
    wout_view = wout_ext.rearrange("(c p) f -> p c f", p=P)   # [128, IC, 1024]
    for m in range(D // P):
        w_f = wstage.tile([P, IC, P], F32, tag="wf", name=f"wf_o_{m}")
        nc.sync.dma_start(w_f, wout_view[:, :, m * P:(m + 1) * P])
        nc.vector.tensor_copy(wout[:, :, m * P:(m + 1) * P], w_f)

    if DEBUG_TAPS:
        for nm, tl in [("dbg_xT", xT), ("dbg_qT", qT), ("dbg_kT", kT),
                       ("dbg_v", v_sb), ("dbg_oT", oT)]:
            dbg = nc.dram_tensor(nm, list(tl.shape), BF16,
                                 kind="ExternalOutput").ap()
            nc.sync.dma_start(dbg, tl[:])

    # ---- output projection + bias ----
    for t in range(NQ // P):          # 8 row chunks of the output
        for fh in range(2):           # two 512-wide column halves
            ps = psum_univ.tile([P, 512], F32, tag="u512", name=f"pout_{t}_{fh}")
            for c in range(IC):
                nc.tensor.matmul(
                    ps, oT[:, c, t * P:(t + 1) * P],
                    wout[:, c, fh * 512:(fh + 1) * 512],
                    start=(c == 0), stop=False,
                )
            nc.tensor.matmul(
                ps, ones_lhsT, bout_bf[:, fh * 512:(fh + 1) * 512],
                start=False, stop=True,
            )
            o_sb = outst.tile([P, 512], F32, tag="osb")
            nc.vector.tensor_copy(o_sb, ps)
            nc.gpsimd.dma_start(
                out_ext[t * P:(t + 1) * P, fh * 512:(fh + 1) * 512], o_sb,
            )


def build():
    nc = bacc.Bacc("TRN2", target_bir_lowering=False, debug=False,
                   num_devices=NCORES)
    x_ext = nc.dram_tensor("x", [N, D], F32, kind="ExternalInput").ap()
    wqkv_ext = nc.dram_tensor("w_qkv", [D, 3 * INNER], F32, kind="ExternalInput").ap()
    wout_ext = nc.dram_tensor("w_out", [INNER, D], F32, kind="ExternalInput").ap()
    bout_ext = nc.dram_tensor("b_out", [1, D], F32, kind="ExternalInput").ap()
    out_ext = nc.dram_tensor("out", [NQ, D], F32, kind="ExternalOutput").ap()

    with tile.TileContext(nc) as tc:
        with ExitStack() as ctx:
            _build_body(ctx, tc, out_ext, x_ext, wqkv_ext, wout_ext, bout_ext)
    nc.compile()
    return nc


_NC_CACHE = None


def _get_nc():
    global _NC_CACHE
    if _NC_CACHE is None:
        _NC_CACHE = build()
    return _NC_CACHE


def make_in_maps(x, W_qkv, W_out, b_out):
    x = np.ascontiguousarray(np.asarray(x, dtype=np.float32))
    W_qkv = np.ascontiguousarray(np.asarray(W_qkv, dtype=np.float32))
    W_out = np.ascontiguousarray(np.asarray(W_out, dtype=np.float32))
    b_out = np.ascontiguousarray(np.asarray(b_out, dtype=np.float32)).reshape(1, D)
    in_maps = []
    for core in range(NCORES):
        bi, s = core // 2, core % 2
        xb = x[bi]
        if s == 1:  # rotate so this core's queries are rows 0:NQ
            xb = np.concatenate([xb[NQ:], xb[:NQ]], axis=0)
        in_maps.append({
            "x": np.ascontiguousarray(xb),
            "w_qkv": W_qkv,
            "w_out": W_out,
            "b_out": b_out,
        })
    return in_maps


def assemble(outs):
    full = np.empty((B, N, D), np.float32)
    for core in range(NCORES):
        bi, s = core // 2, core % 2
        full[bi, s * NQ:(s + 1) * NQ] = outs[core]
    return full


def kernel(x, W_qkv, W_out, b_out):
    from concourse.bass_utils import run_bass_kernel_spmd

    nc = _get_nc()
    in_maps = make_in_maps(x, W_qkv, W_out, b_out)
    res = run_bass_kernel_spmd(nc, in_maps, core_ids=list(range(NCORES)))
    return assemble([r["out"] for r in res.results])
